# Initial kernel scaffold
#
"""Trainium2 Bass kernel for batched dot-product attention.

Problem: query/keys/values [4, 4096, 64] fp32 -> softmax(Q K^T / 8) V.

Sharding: 8 cores, data parallel. Core c handles batch c//2, query rows
(c%2)*2048 : (c%2+1)*2048, and needs full K/V of its batch. Each core runs
an identical program (SPMD) on its own shard.

Per-core algorithm (flash-attention-like, no max subtraction -- scores are
~N(0,1) after the 1/sqrt(64) scale so exp() cannot overflow):
  - K^T [64, 4096] and Q^T [64, 2048] built via PE transposes (contraction
    dim d must sit on partitions for the QK^T matmul).
  - V' = [V | ones] chunks [128, 65]: the ones column makes the second
    matmul emit the softmax denominator as row 64 of the output.
  - For each 512-wide q tile: 32 matmuls produce scoresT [k=128, q=512]
    chunks in PSUM; ScalarE applies exp (scale folded in); second matmul
    accumulates V'^T @ P into out^T [65, 512]; epilogue transposes back to
    [q, 65], divides by the denominator column and DMAs out.
"""

import math

import numpy as np

import concourse.bass as bass
import concourse.tile as tile
from concourse import mybir
from concourse.masks import make_identity

FP32 = mybir.dt.float32

# Production shard dims
B, LQ_FULL, LK, D = 4, 4096, 4096, 64
N_CORES = 8
LQ = LQ_FULL * B // N_CORES  # 2048 query rows per core


def emit_attention(tc, q, k, v, o, LQ, LK, D, QTW):
    """Emit the per-core attention program into TileContext tc.

    q: [LQ, D], k: [LK, D], v: [LK, D] DRAM inputs; o: [LQ, D] DRAM output.
    QTW: q-tile width (free dim of the scores matmul), <= 512 for fp32.
    """
    nc = tc.nc
    NKC = LK // 128   # key chunks
    NQC = LQ // 128   # query chunks (for transposes)
    NQT = LQ // QTW   # q tiles
    DP = D + 1        # V width with the ones column
    SCALE = 1.0 / math.sqrt(D)
    Exp = mybir.ActivationFunctionType.Exp

    from contextlib import ExitStack

    with ExitStack() as ctx:
        const = ctx.enter_context(tc.tile_pool(name="const", bufs=1))
        kq = ctx.enter_context(tc.tile_pool(name="kq", bufs=1))
        loadp = ctx.enter_context(tc.tile_pool(name="load", bufs=4))
        pp = ctx.enter_context(tc.tile_pool(name="p", bufs=4))
        outp = ctx.enter_context(tc.tile_pool(name="outs", bufs=4))
        ps_scores = ctx.enter_context(
            tc.tile_pool(name="ps_scores", bufs=2, space="PSUM")
        )
        ps_acc = ctx.enter_context(tc.tile_pool(name="ps_acc", bufs=2, space="PSUM"))
        ps_tr = ctx.enter_context(tc.tile_pool(name="ps_tr", bufs=2, space="PSUM"))

        ident = const.tile([128, 128], FP32)
        make_identity(nc, ident[:])

        # V' = [V | 1] chunks, [128, DP] each, packed side by side.
        vp = const.tile([128, NKC * DP], FP32)
        for c in range(NKC):
            nc.sync.dma_start(vp[:, c * DP : c * DP + D], v[c * 128 : (c + 1) * 128, :])
            nc.gpsimd.memset(vp[:, c * DP + D : (c + 1) * DP], 1.0)

        # K^T via PE transposes of natural-layout chunks.
        kt = kq.tile([D, LK], FP32)
        for c in range(NKC):
            lt = loadp.tile([128, D], FP32, tag="load")
            nc.sync.dma_start(lt[:], k[c * 128 : (c + 1) * 128, :])
            tr = ps_scores.tile([D, 128], FP32, tag="scores")
            nc.tensor.transpose(tr[:], lt[:], ident[:])
            nc.vector.tensor_copy(kt[:, c * 128 : (c + 1) * 128], tr[:])

        # Q^T likewise.
        qt = kq.tile([D, LQ], FP32)
        for c in range(NQC):
            lt = loadp.tile([128, D], FP32, tag="load")
            nc.sync.dma_start(lt[:], q[c * 128 : (c + 1) * 128, :])
            tr = ps_scores.tile([D, 128], FP32, tag="scores")
            nc.tensor.transpose(tr[:], lt[:], ident[:])
            nc.vector.tensor_copy(qt[:, c * 128 : (c + 1) * 128], tr[:])

        for ti in range(NQT):
            acc = ps_acc.tile([DP, QTW], FP32)
            npair = NKC // 2
            for pi in range(npair):
                c0, c1 = 2 * pi, 2 * pi + 1
                qs = qt[:, ti * QTW : (ti + 1) * QTW]
                s = ps_scores.tile([128, 2 * QTW], FP32, tag="scores")
                nc.tensor.matmul(
                    s[:, :QTW], kt[:, c0 * 128 : (c0 + 1) * 128], qs,
                    start=True, stop=True,
                )
                nc.tensor.matmul(
                    s[:, QTW:], kt[:, c1 * 128 : (c1 + 1) * 128], qs,
                    start=True, stop=True,
                )
                p = pp.tile([128, 2 * QTW], FP32, tag="p")
                nc.scalar.activation(p[:], s[:], Exp, scale=SCALE)
                nc.tensor.matmul(
                    acc[:], vp[:, c0 * DP : (c0 + 1) * DP], p[:, :QTW],
                    start=(pi == 0), stop=False,
                )
                nc.tensor.matmul(
                    acc[:], vp[:, c1 * DP : (c1 + 1) * DP], p[:, QTW:],
                    start=False, stop=(pi == npair - 1),
                )

            # Epilogue: transpose out^T back to [q, DP], divide by denominator.
            accs = outp.tile([DP, QTW], FP32, tag="accs")
            nc.vector.tensor_copy(accs[:], acc[:])
            for j in range(QTW // 128):
                tr = ps_tr.tile([128, DP], FP32)
                nc.tensor.transpose(tr[:], accs[:, j * 128 : (j + 1) * 128],
                                    ident[:DP, :DP])
                rc = outp.tile([128, 1], FP32, tag="rc")
                nc.vector.reciprocal(rc[:], tr[:, D:DP])
                ot = outp.tile([128, D], FP32, tag="ot")
                nc.vector.tensor_scalar_mul(ot[:], tr[:, :D], rc[:])
                r0 = ti * QTW + j * 128
                nc.sync.dma_start(o[r0 : r0 + 128, :], ot[:])


_BUILT = None


def _build():
    global _BUILT
    if _BUILT is None:
        nc = bass.Bass("TRN2", target_bir_lowering=False, debug=False)
        q = nc.dram_tensor("q", [LQ, D], FP32, kind="ExternalInput")
        k = nc.dram_tensor("k", [LK, D], FP32, kind="ExternalInput")
        v = nc.dram_tensor("v", [LK, D], FP32, kind="ExternalInput")
        o = nc.dram_tensor("o", [LQ, D], FP32, kind="ExternalOutput")
        with tile.TileContext(nc) as tc:
            emit_attention(tc, q[:], k[:], v[:], o[:], LQ, LK, D, QTW=512)
        _BUILT = nc
    return _BUILT


def _shard_inputs(query, keys, values):
    in_maps = []
    for c in range(N_CORES):
        b, h = c // 2, c % 2
        in_maps.append({
            "q": np.ascontiguousarray(query[b, h * LQ : (h + 1) * LQ, :],
                                      dtype=np.float32),
            "k": np.ascontiguousarray(keys[b], dtype=np.float32),
            "v": np.ascontiguousarray(values[b], dtype=np.float32),
        })
    return in_maps


def run_sharded(query, keys, values, trace=False):
    """Run on 8 cores; returns (full_output, BassKernelResults)."""
    from concourse.bass_utils import run_bass_kernel_spmd

    nc = _build()
    in_maps = _shard_inputs(query, keys, values)
    res = run_bass_kernel_spmd(nc, in_maps, list(range(N_CORES)), trace=trace)
    out = np.empty((B, LQ_FULL, D), np.float32)
    for c in range(N_CORES):
        b, h = c // 2, c % 2
        out[b, h * LQ : (h + 1) * LQ, :] = res.results[c]["o"]
    return out, res


def kernel(query, keys, values):
    out, _ = run_sharded(np.asarray(query), np.asarray(keys), np.asarray(values))
    return out


# revision 10
# speedup vs baseline: 1.3765x; 1.3765x over previous
"""Trainium2 Bass kernel for batched dot-product attention.

Problem: query/keys/values [4, 4096, 64] fp32 -> softmax(Q K^T / 8) V.

Sharding: 8 cores, data parallel. Core c handles batch c//2, query rows
(c%2)*2048 : (c%2+1)*2048, and needs full K/V of its batch. Each core runs
an identical program (SPMD) on its own shard.

Per-core algorithm (flash-attention-like, no max subtraction -- scores are
~N(0,1) after the 1/sqrt(64) scale so exp() cannot overflow):
  - K^T [64, 4096] and Q^T [64, 2048] built via PE transposes (contraction
    dim d must sit on partitions for the QK^T matmul).
  - V' = [V | ones] chunks [128, 65]: the ones column makes the second
    matmul emit the softmax denominator as row 64 of the output.
  - For each 512-wide q tile: 32 matmuls produce scoresT [k=128, q=512]
    chunks in PSUM; ScalarE applies exp (scale folded in); second matmul
    accumulates V'^T @ P into out^T [65, 512]; epilogue transposes back to
    [q, 65], divides by the denominator column and DMAs out.
"""

import math

import numpy as np

import concourse.bass as bass
import concourse.tile as tile
from concourse import bacc, mybir
from concourse.masks import make_identity

FP32 = mybir.dt.float32

# Production shard dims
B, LQ_FULL, LK, D = 4, 4096, 4096, 64
N_CORES = 8
LQ = LQ_FULL * B // N_CORES  # 2048 query rows per core


def emit_attention(tc, q, k, v, o, LQ, LK, D, QTW):
    """Emit the per-core attention program into TileContext tc.

    q: [LQ, D], k: [LK, D], v: [LK, D] DRAM inputs; o: [LQ, D] DRAM output.
    QTW: q-tile width (free dim of the scores matmul), <= 512 for fp32.
    """
    nc = tc.nc
    NKC = LK // 128   # key chunks
    NQC = LQ // 128   # query chunks (for transposes)
    NQT = LQ // QTW   # q tiles
    DP = D + 1        # V width with the ones column
    SCALE = 1.0 / math.sqrt(D)
    Exp = mybir.ActivationFunctionType.Exp

    from contextlib import ExitStack

    with ExitStack() as ctx:
        const = ctx.enter_context(tc.tile_pool(name="const", bufs=1))
        kq = ctx.enter_context(tc.tile_pool(name="kq", bufs=1))
        loadp = ctx.enter_context(tc.tile_pool(name="load", bufs=1))
        pp = ctx.enter_context(tc.tile_pool(name="p", bufs=4))
        outp = ctx.enter_context(tc.tile_pool(name="outs", bufs=4))
        ps_scores = ctx.enter_context(
            tc.tile_pool(name="ps_scores", bufs=2, space="PSUM")
        )
        ps_acc = ctx.enter_context(tc.tile_pool(name="ps_acc", bufs=2, space="PSUM"))
        ps_tr = ctx.enter_context(tc.tile_pool(name="ps_tr", bufs=2, space="PSUM"))

        ident = const.tile([128, 128], FP32)
        make_identity(nc, ident[:])

        # V' = [V | 1] chunks, [128, DP] each, packed side by side.
        # One DMA for all of V, one strided memset for the ones columns.
        vp = const.tile([128, NKC * DP], FP32)
        vp3 = vp[:].rearrange("p (c w) -> p c w", w=DP)
        nc.sync.dma_start(vp3[:, :, 0:D], v.rearrange("(c p) d -> p c d", p=128))
        nc.gpsimd.memset(vp3[:, :, D:DP], 1.0)

        # Natural-layout K/Q staged chunk-major with one DMA each.
        kn = loadp.tile([128, NKC * D], FP32, tag="kn")
        nc.sync.dma_start(
            kn[:].rearrange("p (c d) -> p c d", d=D),
            k.rearrange("(c p) d -> p c d", p=128),
        )
        qn = loadp.tile([128, NQC * D], FP32, tag="qn")
        nc.sync.dma_start(
            qn[:].rearrange("p (c d) -> p c d", d=D),
            q.rearrange("(c p) d -> p c d", p=128),
        )

        # K^T / Q^T via PE transposes of the natural-layout chunks.
        kt = kq.tile([D, LK], FP32)
        for c in range(NKC):
            tr = ps_scores.tile([D, 128], FP32, tag="scores")
            nc.tensor.transpose(tr[:], kn[:, c * D : (c + 1) * D], ident[:])
            nc.vector.tensor_copy(kt[:, c * 128 : (c + 1) * 128], tr[:])

        qt = kq.tile([D, LQ], FP32)
        for c in range(NQC):
            tr = ps_scores.tile([D, 128], FP32, tag="scores")
            nc.tensor.transpose(tr[:], qn[:, c * D : (c + 1) * D], ident[:])
            nc.vector.tensor_copy(qt[:, c * 128 : (c + 1) * 128], tr[:])

        for ti in range(NQT):
            acc = ps_acc.tile([DP, QTW], FP32)
            npair = NKC // 2
            for pi in range(npair):
                c0, c1 = 2 * pi, 2 * pi + 1
                qs = qt[:, ti * QTW : (ti + 1) * QTW]
                s = ps_scores.tile([128, 2 * QTW], FP32, tag="scores")
                nc.tensor.matmul(
                    s[:, :QTW], kt[:, c0 * 128 : (c0 + 1) * 128], qs,
                    start=True, stop=True,
                )
                nc.tensor.matmul(
                    s[:, QTW:], kt[:, c1 * 128 : (c1 + 1) * 128], qs,
                    start=True, stop=True,
                )
                p = pp.tile([128, 2 * QTW], FP32, tag="p")
                nc.scalar.activation(p[:], s[:], Exp, scale=SCALE)
                nc.tensor.matmul(
                    acc[:], vp[:, c0 * DP : (c0 + 1) * DP], p[:, :QTW],
                    start=(pi == 0), stop=False,
                )
                nc.tensor.matmul(
                    acc[:], vp[:, c1 * DP : (c1 + 1) * DP], p[:, QTW:],
                    start=False, stop=(pi == npair - 1),
                )

            # Epilogue: transpose out^T back to [q, DP], divide by denominator.
            accs = outp.tile([DP, QTW], FP32, tag="accs")
            nc.vector.tensor_copy(accs[:], acc[:])
            for j in range(QTW // 128):
                tr = ps_tr.tile([128, DP], FP32)
                nc.tensor.transpose(tr[:], accs[:, j * 128 : (j + 1) * 128],
                                    ident[:DP, :DP])
                rc = outp.tile([128, 1], FP32, tag="rc")
                nc.vector.reciprocal(rc[:], tr[:, D:DP])
                ot = outp.tile([128, D], FP32, tag="ot")
                nc.vector.tensor_scalar_mul(ot[:], tr[:, :D], rc[:])
                r0 = ti * QTW + j * 128
                nc.sync.dma_start(o[r0 : r0 + 128, :], ot[:])


BF16 = mybir.dt.bfloat16


def emit_attention_v2(tc, q, k, v, o, LQ, LK, D, QTW):
    """v2: bf16 K^T/Q^T via DMA xbar transposes in a packed layout
    (even k-chunks on partitions 0-63, odd on 64-127) + row-tiled
    concurrent scores matmuls + 3-chunk exp groups.

    Requires D == 64 (pair-packing trick) and QTW == 512.
    """
    nc = tc.nc
    assert D == 64 and QTW % 128 == 0
    NKC = LK // 128   # k chunks
    NQC = LQ // 128   # q chunks
    NQT = LQ // QTW
    DP = D + 1
    GRP = 3           # chunks per exp group (3 PSUM banks)
    SCALE = 1.0 / math.sqrt(D)
    Exp = mybir.ActivationFunctionType.Exp

    from contextlib import ExitStack

    with ExitStack() as ctx:
        const = ctx.enter_context(tc.tile_pool(name="const", bufs=1))
        loadp = ctx.enter_context(tc.tile_pool(name="load", bufs=1))
        pp = ctx.enter_context(tc.tile_pool(name="p", bufs=4))
        outp = ctx.enter_context(tc.tile_pool(name="outs", bufs=4))
        ps_scores = ctx.enter_context(
            tc.tile_pool(name="ps_scores", bufs=2, space="PSUM")
        )
        ps_acc = ctx.enter_context(tc.tile_pool(name="ps_acc", bufs=1, space="PSUM"))
        ps_tr = ctx.enter_context(tc.tile_pool(name="ps_tr", bufs=1, space="PSUM"))

        ident = const.tile([128, 128], FP32)
        make_identity(nc, ident[:])

        # V' = [V | 1] chunks (fp32; value path stays fp32).
        vp = const.tile([128, NKC * DP], FP32)
        vp3 = vp[:].rearrange("p (c w) -> p c w", w=DP)
        nc.sync.dma_start(vp3[:, :, 0:D], v.rearrange("(c p) d -> p c d", p=128))
        nc.gpsimd.memset(vp3[:, :, D:DP], 1.0)

        # K: load natural chunk-major, cast to bf16, xbar-transpose pair
        # blocks. Transposing kb[:, j*128:(j+1)*128] (= chunks 2j | 2j+1 side
        # by side) yields K^T of chunk 2j on partitions 0-63 and of chunk
        # 2j+1 on partitions 64-127 -- the packed row-tiling layout.
        kn = loadp.tile([128, NKC * D], FP32, tag="kn")
        nc.sync.dma_start(
            kn[:].rearrange("p (c d) -> p c d", d=D),
            k.rearrange("(c p) d -> p c d", p=128),
        )
        kb = loadp.tile([128, NKC * D], BF16, tag="kb")
        nc.vector.tensor_copy(kb[:], kn[:])
        kt2 = const.tile([128, NKC * D], BF16)
        for j in range(NKC // 2):
            nc.sync.dma_start(
                kt2[:, j * 128 : (j + 1) * 128],
                kb[:, j * 128 : (j + 1) * 128],
                transpose=True,
            )

        # Q: same transpose trick into alternating halves, then rearrange
        # into qt2r with Q^T replicated on both partition halves.
        qn = loadp.tile([128, NQC * D], FP32, tag="qn")
        nc.sync.dma_start(
            qn[:].rearrange("p (c d) -> p c d", d=D),
            q.rearrange("(c p) d -> p c d", p=128),
        )
        qb = loadp.tile([128, NQC * D], BF16, tag="qb")
        nc.vector.tensor_copy(qb[:], qn[:])
        qt_alt = loadp.tile([128, NQC * D], BF16, tag="qt_alt")
        for j in range(NQC // 2):
            nc.sync.dma_start(
                qt_alt[:, j * 128 : (j + 1) * 128],
                qb[:, j * 128 : (j + 1) * 128],
                transpose=True,
            )
        qt2r = const.tile([128, LQ], BF16)
        nblk = NQC // 2
        for parity in range(2):
            src = qt_alt[parity * 64 : (parity + 1) * 64, :].rearrange(
                "p (j c) -> p j c", c=128
            )
            for half in range(2):
                dst = qt2r[half * 64 : (half + 1) * 64, parity * 128 :]
                dst = dst.rearrange("p (j c) -> p j c", c=256)[:, :, 0:128]
                nc.sync.dma_start(dst, src)

        for ti in range(NQT):
            acc = ps_acc.tile([DP, QTW], FP32)
            ngrp = (NKC + GRP - 1) // GRP
            s_tiles = [None] * ngrp
            filled = [0] * ngrp
            qsl = slice(ti * QTW, (ti + 1) * QTW)

            def consume_group(g):
                gsz = min(GRP, NKC - g * GRP)
                p = pp.tile([128, GRP * QTW], FP32, tag="p")
                nc.scalar.activation(
                    p[:, : gsz * QTW], s_tiles[g][:, : gsz * QTW], Exp, scale=SCALE
                )
                for s in range(gsz):
                    c = g * GRP + s
                    nc.tensor.matmul(
                        acc[:],
                        vp[:, c * DP : (c + 1) * DP],
                        p[:, s * QTW : (s + 1) * QTW],
                        start=(c == 0),
                        stop=(c == NKC - 1),
                    )

            for c in range(NKC):
                g, slot = divmod(c, GRP)
                if s_tiles[g] is None:
                    s_tiles[g] = ps_scores.tile([128, GRP * QTW], FP32, tag="scores")
                half, j = c % 2, c // 2
                nc.tensor.matmul(
                    s_tiles[g][:, slot * QTW : (slot + 1) * QTW],
                    kt2[half * 64 : (half + 1) * 64, j * 128 : (j + 1) * 128],
                    qt2r[half * 64 : (half + 1) * 64, qsl],
                    start=True,
                    stop=True,
                )
                filled[g] += 1
                if filled[g] == min(GRP, NKC - g * GRP):
                    consume_group(g)

            # Epilogue: transpose out^T back to [q, DP], divide by denom.
            accs = outp.tile([DP, QTW], FP32, tag="accs")
            nc.vector.tensor_copy(accs[:], acc[:])
            for j in range(QTW // 128):
                tr = ps_tr.tile([128, DP], FP32)
                nc.tensor.transpose(tr[:], accs[:, j * 128 : (j + 1) * 128],
                                    ident[:DP, :DP])
                rc = outp.tile([128, 1], FP32, tag="rc")
                nc.vector.reciprocal(rc[:], tr[:, D:DP])
                ot = outp.tile([128, D], FP32, tag="ot")
                nc.vector.tensor_scalar_mul(ot[:], tr[:, :D], rc[:])
                r0 = ti * QTW + j * 128
                nc.sync.dma_start(o[r0 : r0 + 128, :], ot[:])


_BUILT = {}

EMITTERS = {"v1": emit_attention, "v2": emit_attention_v2}
DEFAULT_VARIANT = "v1"


def _build(repeat=1, variant=None):
    """Build the per-core module. repeat>1 wraps the body in a hardware
    For_i loop (used only for on-device timing; grading uses repeat=1)."""
    variant = variant or DEFAULT_VARIANT
    key = (repeat, variant)
    if key not in _BUILT:
        emitter = EMITTERS[variant]
        nc = bacc.Bacc("TRN2", target_bir_lowering=False, debug=False)
        q = nc.dram_tensor("q", [LQ, D], FP32, kind="ExternalInput")
        k = nc.dram_tensor("k", [LK, D], FP32, kind="ExternalInput")
        v = nc.dram_tensor("v", [LK, D], FP32, kind="ExternalInput")
        o = nc.dram_tensor("o", [LQ, D], FP32, kind="ExternalOutput")
        with tile.TileContext(nc) as tc:
            if repeat == 1:
                emitter(tc, q[:], k[:], v[:], o[:], LQ, LK, D, QTW=512)
            else:
                engines = [
                    mybir.EngineType.PE,
                    mybir.EngineType.Activation,
                    mybir.EngineType.DVE,
                    mybir.EngineType.SP,
                    mybir.EngineType.Pool,
                ]
                with tc.For_i(0, repeat, 1, hint_engines=tuple(engines)):
                    emitter(tc, q[:], k[:], v[:], o[:], LQ, LK, D, QTW=512)
        nc.compile()
        _BUILT[key] = nc
    return _BUILT[key]


def _shard_inputs(query, keys, values):
    in_maps = []
    for c in range(N_CORES):
        b, h = c // 2, c % 2
        in_maps.append({
            "q": np.ascontiguousarray(query[b, h * LQ : (h + 1) * LQ, :],
                                      dtype=np.float32),
            "k": np.ascontiguousarray(keys[b], dtype=np.float32),
            "v": np.ascontiguousarray(values[b], dtype=np.float32),
        })
    return in_maps


def run_sharded(query, keys, values, trace=False, repeat=1, variant=None):
    """Run on 8 cores; returns (full_output, BassKernelResults)."""
    from concourse.bass_utils import run_bass_kernel_spmd

    nc = _build(repeat, variant)
    in_maps = _shard_inputs(query, keys, values)
    res = run_bass_kernel_spmd(nc, in_maps, list(range(N_CORES)), trace=trace)
    out = np.empty((B, LQ_FULL, D), np.float32)
    for c in range(N_CORES):
        b, h = c // 2, c % 2
        out[b, h * LQ : (h + 1) * LQ, :] = res.results[c]["o"]
    return out, res


def kernel(query, keys, values):
    out, _ = run_sharded(np.asarray(query), np.asarray(keys), np.asarray(values))
    return out
